# revision 11
# baseline (speedup 1.0000x reference)
"""Trainium2 Bass kernel for MultiHeadSelfAttention with RoPE.

Problem: x[2, 2048, 1024] @ W_qkv[1024, 3072] -> rope(q,k) -> softmax(q k^T/8) v
         -> out @ W_out[1024, 1024].

Sharding (8 cores): batch (2-way) x head-group (4-way, 4 heads each).
Each core computes a partial output [2048, 1024] = attnout_heads @ W_out_rows;
host sums the 4 head-group partials per batch.

v2 design (vs baseline): single fused pipeline instead of serial phases.
 - x is DMA'd in 512-column chunks; k/v/q projections start on chunk 0
   while later chunks stream in.
 - Attention inner loop is ACT(exp)-bound (~1.15us per sk tile); leftover
   projection + output-projection matmuls are interleaved ("fill work")
   into the PE slack of the attention loop.
 - Scores use K=64 matmuls on packed q/k tiles (measured same throughput
   as K=128 on f32r/bf16; no zero-padded q needed).
 - All attention operands (kT, q, exp(attn), v, att_o, W_out) are bf16:
   same PE rate, half SBUF, 2x DVE. Projections stay f32r from f32 x/W.
 - Softmax denominator rides as a ones-column in the attnv lhsT (row 64
   of the psum accumulator); normalization = reciprocal of that row,
   K=1 ones-outer-product broadcast matmul, one DVE multiply.
 - PSUM: scores double-buffered (2x2 banks) + attnv accumulator (2 banks)
   + a shared 1-bank ring (proj/rope/broadcast/outproj) x2.
"""

import sys

if "/opt/trn_rl_repo" not in sys.path:
    sys.path.insert(0, "/opt/trn_rl_repo")

import numpy as np

B, S, E = 2, 2048, 1024
ATT = 1024
H = 16
D = 64
HG = 4            # head groups (cores per batch)
HPG = H // HG     # heads per core = 4
PAIRS = HPG // 2  # head pairs per core = 2
ROPE_THETA = 10000.0
N_CORES = 8

EK = E // 128     # 8 contraction tiles over embedding dim
NCH = S // 512    # 4 x/proj column chunks
N_SK = S // 128   # 16 sk tiles
SQ_CHUNK = 1024   # q chunk for attention blocks
N_CH = S // SQ_CHUNK  # 2 attention chunks

_BUILT = {}


def _build_program():
    import concourse.bacc as bacc
    import concourse.tile as tile
    import concourse.mybir as mybir

    f32 = mybir.dt.float32
    f32r = mybir.dt.float32r
    bf16 = mybir.dt.bfloat16
    AF = mybir.ActivationFunctionType

    nc = bacc.Bacc(
        "TRN2",
        target_bir_lowering=False,
        debug=False,
        enable_asserts=False,
        num_devices=N_CORES,
    )

    xT = nc.dram_tensor("xT", [E, S], f32r, kind="ExternalInput").ap()
    w_qk = nc.dram_tensor("w_qk", [E, 2 * HPG * D], f32r, kind="ExternalInput").ap()
    w_v = nc.dram_tensor("w_v", [E, HPG * D], f32r, kind="ExternalInput").ap()
    w_o = nc.dram_tensor("w_o", [HPG * D, E], bf16, kind="ExternalInput").ap()
    cos_t = nc.dram_tensor("cos_t", [128, S], f32, kind="ExternalInput").ap()
    sin_t = nc.dram_tensor("sin_t", [128, S], f32, kind="ExternalInput").ap()
    mswap = nc.dram_tensor("mswap", [128, 128], f32r, kind="ExternalInput").ap()
    out = nc.dram_tensor("out", [S, E], f32, kind="ExternalOutput").ap()

    with tile.TileContext(nc) as tc:
        with (
            tc.tile_pool(name="const", bufs=1) as constp,
            tc.tile_pool(name="persist", bufs=1) as pers,
            tc.tile_pool(name="xt", bufs=1) as xtp,
            tc.tile_pool(name="wqk", bufs=1) as wqkp,
            tc.tile_pool(name="wv", bufs=1) as wvp,
            tc.tile_pool(name="trig", bufs=1) as trigp,
            tc.tile_pool(name="raw", bufs=3) as rawp,
            tc.tile_pool(name="tt", bufs=4) as ttp,
            tc.tile_pool(name="exp", bufs=3) as expp,
            tc.tile_pool(name="oa", bufs=3) as oap,
            tc.tile_pool(name="ev", bufs=3) as evp,
            tc.tile_pool(name="sA", bufs=2, space="PSUM") as sAp,
            tc.tile_pool(name="oT", bufs=1, space="PSUM") as oTp,
            tc.tile_pool(name="r1", bufs=2, space="PSUM") as r1p,
        ):
            msw = constp.tile([128, 128], f32r, tag="msw")
            # ones row at partition 64: matmul requires lhsT and rhs to
            # share a base partition, and the denom row sits at row 64.
            # (f32r/bf16 memsets are invalid ISA; memset f32 and copy.)
            ones_f32 = constp.tile([128, 64], f32, tag="ones_f32")
            nc.gpsimd.memset(ones_f32[:], 1.0)
            ones1 = constp.tile([65, 64], f32r, tag="ones1")
            nc.vector.tensor_copy(ones1[64:65, :], ones_f32[64:65, :])

            # persistent attention tensors (all bf16)
            kT = [pers.tile([128, S], bf16, tag=f"kT{g}", name=f"kT{g}") for g in range(PAIRS)]
            qp = [pers.tile([128, S], bf16, tag=f"qp{g}", name=f"qp{g}") for g in range(PAIRS)]
            # v + ones-aug col per (pair g, head hh): cols 130g+65hh .. +65
            v_c = pers.tile([128, N_SK, 4 * 65], bf16, tag="vc")
            for hcol in range(4):
                nc.vector.tensor_copy(
                    v_c[:, :, 65 * hcol + 64], ones_f32[:, 0:N_SK]
                )
            att_o = [pers.tile([128, S], bf16, tag=f"ao{g}", name=f"ao{g}") for g in range(PAIRS)]
            wo_sb = [pers.tile([128, E], bf16, tag=f"wo{g}", name=f"wo{g}") for g in range(PAIRS)]

            cos_sb = trigp.tile([128, S], f32, tag="cos")
            sin_sb = trigp.tile([128, S], f32, tag="sin")

            # ---------------- DMA (ordered; x chunked so compute starts early)
            wqk_sb = []
            for e in range(EK):
                t = wqkp.tile([128, 2 * HPG * D], f32r, tag=f"wqk{e}", name=f"wqk{e}")
                nc.sync.dma_start(t[:], w_qk[128 * e : 128 * (e + 1), :])
                wqk_sb.append(t)
            xt_sb = [[None] * NCH for _ in range(EK)]
            for c in range(NCH):
                csl = slice(512 * c, 512 * (c + 1))
                for e in range(EK):
                    t = xtp.tile([128, 512], f32r, tag=f"xt{e}_{c}", name=f"xt{e}_{c}")
                    nc.sync.dma_start(t[:], xT[128 * e : 128 * (e + 1), csl])
                    xt_sb[e][c] = t
                nc.sync.dma_start(cos_sb[:, csl], cos_t[:, csl])
                nc.sync.dma_start(sin_sb[:, csl], sin_t[:, csl])
                if c == 0:
                    nc.sync.dma_start(msw[:], mswap[:])
                    wv_sb = []
                    for e in range(EK):
                        tv = wvp.tile([128, HPG * D], f32r, tag=f"wv{e}", name=f"wv{e}")
                        nc.sync.dma_start(tv[:], w_v[128 * e : 128 * (e + 1), :])
                        wv_sb.append(tv)
                if c == 1:
                    for g in range(PAIRS):
                        nc.sync.dma_start(wo_sb[g][:], w_o[128 * g : 128 * (g + 1), :])

            # ---------------- projection + rope helpers ----------------
            rope_pend = []

            def rope_tail():
                if not rope_pend:
                    return
                (g_, ti, sl, raw) = rope_pend.pop(0)
                rp = r1p.tile([128, 512], f32, tag="r1", name=f"rot{g_}_{ti}_{sl.start}")
                nc.tensor.matmul(rp[:], msw[:], raw[:], start=True, stop=True)
                t2 = ttp.tile([128, 512], f32, tag="tt")
                nc.vector.tensor_mul(t2[:], raw[:], cos_sb[:, sl])
                t1 = ttp.tile([128, 512], f32, tag="tt")
                nc.vector.tensor_mul(t1[:], rp[:], sin_sb[:, sl])
                if ti == 1:
                    nc.vector.tensor_add(kT[g_][:, sl], t1[:], t2[:])
                else:
                    nc.gpsimd.tensor_tensor(
                        qp[g_][:, sl], t1[:], t2[:], mybir.AluOpType.add
                    )

            def qk_items(g, ti, c):
                """Items projecting+roping one 512-col chunk of q (ti=0) or
                k (ti=1) of pair g: 8 single-matmul items + raw-copy + rope."""
                coff = ti * HPG * D + 128 * g
                sl = slice(512 * c, 512 * (c + 1))
                cell = {}

                def mk_mm(e):
                    def it():
                        if e == 0:
                            cell["pp"] = r1p.tile(
                                [128, 512], f32, tag="r1", name=f"pj{g}_{ti}_{c}"
                            )
                        nc.tensor.matmul(
                            cell["pp"][:],
                            wqk_sb[e][:, coff : coff + 128],
                            xt_sb[e][c][:],
                            start=(e == 0),
                            stop=(e == EK - 1),
                        )
                    return it

                def raw_it():
                    raw = rawp.tile([128, 512], f32r, tag="raw")
                    nc.vector.tensor_copy(raw[:], cell["pp"][:])
                    rope_pend.append((g, ti, sl, raw))

                return [mk_mm(e) for e in range(EK)] + [raw_it, rope_tail]

            def qk_chunk(g, ti, c):
                for it in qk_items(g, ti, c):
                    it()

            def v_st(st):
                vp_ps = r1p.tile([128, 256], f32, tag="r1", name=f"vps{st}")
                c, sub = divmod(st, 4)
                for e in range(EK):
                    nc.tensor.matmul(
                        vp_ps[:],
                        xt_sb[e][c][:, 128 * sub : 128 * (sub + 1)],
                        wv_sb[e][:],
                        start=(e == 0),
                        stop=(e == EK - 1),
                    )
                for h in range(4):
                    nc.vector.tensor_copy(
                        v_c[:, st, 65 * h : 65 * h + 64],
                        vp_ps[:, 64 * h : 64 * h + 64],
                    )

            # ---------------- preamble (paced by x-chunk arrival) ----------
            # Minimum before attention can start: kT pair 0 (all chunks),
            # v (all, consumed 1 tile/iter by attnv), q pair 0 cols 0:1024.
            for c in range(NCH):
                qk_chunk(0, 1, c)            # k pair 0
                for st in range(4 * c, 4 * c + 4):
                    v_st(st)
                if c < 2:
                    qk_chunk(0, 0, c)        # q pair 0 chunks 0,1
            rope_tail()                      # flush pending rope
            rope_tail()

            # fill work queue: pumped a few items per attention iteration
            fill = []
            for (g_, ti_, c_) in [(0, 0, 2), (0, 0, 3),
                                  (1, 1, 0), (1, 1, 1), (1, 1, 2), (1, 1, 3),
                                  (1, 0, 0), (1, 0, 1), (1, 0, 2), (1, 0, 3)]:
                fill.extend(qk_items(g_, ti_, c_))
            fill.append(rope_tail)  # drain last pending rope
            fill.append(rope_tail)

            def out_st(st, n):
                ssl = slice(128 * st, 128 * (st + 1))
                nsl = slice(512 * n, 512 * (n + 1))
                op = r1p.tile([128, 512], f32, tag="r1", name=f"op{st}_{n}")
                for g in range(PAIRS):
                    nc.tensor.matmul(
                        op[:],
                        att_o[g][:, ssl],
                        wo_sb[g][:, nsl],
                        start=(g == 0),
                        stop=(g == PAIRS - 1),
                    )
                ev = evp.tile([128, 512], f32, tag="ev")
                nc.vector.tensor_copy(ev[:], op[:])
                nc.sync.dma_start(out[ssl, nsl], ev[:])

            # ---------------- attention (ACT-bound; PE slack runs fill) ----
            def attention_block(g, ch, hh):
                base = 130 * g + 65 * hh
                rows = slice(64 * hh, 64 * hh + 64)
                csl = slice(SQ_CHUNK * ch, SQ_CHUNK * (ch + 1))
                oT = oTp.tile([65, SQ_CHUNK], f32, tag="oT", name=f"oT{g}_{ch}_{hh}")
                exps = []

                def attnv(j):
                    e_t = exps[j]
                    for n in range(2):
                        nsl = slice(512 * n, 512 * (n + 1))
                        nc.tensor.matmul(
                            oT[:, nsl],
                            v_c[:, j, base : base + 65],
                            e_t[:, nsl],
                            start=(j == 0),
                            stop=(j == N_SK - 1),
                        )

                for sk in range(N_SK):
                    sksl = slice(128 * sk, 128 * (sk + 1))
                    sA = sAp.tile([128, SQ_CHUNK], f32, tag="sA",
                                  name=f"sA{g}_{ch}_{hh}_{sk}")
                    for n in range(2):
                        nsl = slice(512 * n, 512 * (n + 1))
                        gsl = slice(SQ_CHUNK * ch + 512 * n,
                                    SQ_CHUNK * ch + 512 * (n + 1))
                        nc.tensor.matmul(
                            sA[:, nsl],
                            kT[g][rows, sksl],
                            qp[g][rows, gsl],
                            start=True,
                            stop=True,
                        )
                    e_t = expp.tile([128, SQ_CHUNK], bf16, tag="eA")
                    nc.scalar.activation(e_t[:], sA[:], AF.Exp, scale=0.125)
                    exps.append(e_t)
                    if sk > 0:
                        attnv(sk - 1)
                    npump = 2 if nblk[0] < 2 else 1
                    for _ in range(npump):
                        if fill:
                            fill.pop(0)()
                attnv(N_SK - 1)
                nblk[0] += 1

                # normalize: denom in oT row 64. Copy to SBUF promptly
                # (frees oT for the next block); the broadcast/reciprocal/
                # multiply are queued as fill work so the block boundary
                # doesn't serialize the scores/exp pipeline.
                oA = oap.tile([65, SQ_CHUNK], f32r, tag="oA")
                nc.vector.tensor_copy(oA[:], oT[:])

                def norm_n(n, oA=oA, g=g, ch=ch, hh=hh, rows=rows):
                    nsl = slice(512 * n, 512 * (n + 1))
                    rb = sAp.tile([64, 512], f32, tag="sA",
                                  name=f"rb{g}_{ch}_{hh}_{n}")
                    nc.tensor.matmul(rb[:], ones1[64:65, :], oA[64:65, nsl],
                                     start=True, stop=True)
                    rbr = oap.tile([64, 512], f32, tag="rbr")
                    nc.vector.reciprocal_approx_fast(rbr[:], rb[:])
                    nc.vector.tensor_mul(
                        att_o[g][rows, SQ_CHUNK * ch + 512 * n :
                                 SQ_CHUNK * ch + 512 * (n + 1)],
                        oA[0:64, nsl],
                        rbr[:],
                    )

                norm_n(0)
                norm_n(1)

            iters_left = [8 * N_SK]

            nblk = [0]
            for g in range(PAIRS):
                for ch in range(N_CH):
                    for hh in range(2):
                        attention_block(g, ch, hh)
                    if g == 1:
                        for st in range(8 * ch, 8 * ch + 8):
                            fill.append(lambda st=st: out_st(st, 0))
                            fill.append(lambda st=st: out_st(st, 1))

            while fill:
                fill.pop(0)()

    nc.compile()
    return nc


def _get_program():
    if "nc" not in _BUILT:
        _BUILT["nc"] = _build_program()
    return _BUILT["nc"]


def _host_inputs(x, W_qkv, W_out):
    """Build the 8 per-core input maps."""
    import ml_dtypes

    f = np.float32
    x = np.asarray(x, dtype=f)
    W_qkv = np.asarray(W_qkv, dtype=f)
    W_out = np.asarray(W_out, dtype=f)

    inv_freq = 1.0 / (ROPE_THETA ** (np.arange(0, D, 2, dtype=np.float64) / D))
    p = np.arange(128)
    freq_row = inv_freq[(p % D) // 2]  # [128]
    ang = freq_row[:, None] * np.arange(S, dtype=np.float64)[None, :]  # [128, S]
    cos_t = np.cos(ang).astype(f)
    sign = np.where(p % 2 == 0, -1.0, 1.0)[:, None]
    sin_t = (np.sin(ang) * sign).astype(f)

    msw = np.zeros((128, 128), dtype=f)
    msw[p, p ^ 1] = 1.0

    maps = []
    for core in range(N_CORES):
        b, hg = divmod(core, HG)
        hs = [HPG * hg + i for i in range(HPG)]
        w_qk = np.concatenate(
            [W_qkv[:, h * D : (h + 1) * D] for h in hs]
            + [W_qkv[:, ATT + h * D : ATT + (h + 1) * D] for h in hs],
            axis=1,
        )
        w_v = np.concatenate(
            [W_qkv[:, 2 * ATT + h * D : 2 * ATT + (h + 1) * D] for h in hs], axis=1
        )
        w_o = np.concatenate([W_out[h * D : (h + 1) * D, :] for h in hs], axis=0)
        maps.append(
            {
                "xT": np.ascontiguousarray(x[b].T),
                "w_qk": np.ascontiguousarray(w_qk),
                "w_v": np.ascontiguousarray(w_v),
                "w_o": np.ascontiguousarray(w_o).astype(ml_dtypes.bfloat16),
                "cos_t": cos_t,
                "sin_t": sin_t,
                "mswap": msw,
            }
        )
    return maps


def kernel(x, W_qkv, W_out):
    from concourse.bass_utils import run_bass_kernel_spmd

    nc = _get_program()
    maps = _host_inputs(x, W_qkv, W_out)
    res = run_bass_kernel_spmd(nc, maps, core_ids=list(range(N_CORES)))
    out = np.zeros((B, S, E), dtype=np.float32)
    for core in range(N_CORES):
        b = core // HG
        out[b] += res.results[core]["out"]
    return out


# revision 15
# speedup vs baseline: 1.1253x; 1.1253x over previous
"""Trainium2 Bass kernel for MultiHeadSelfAttention with RoPE.

Problem: x[2, 2048, 1024] @ W_qkv[1024, 3072] -> rope(q,k) -> softmax(q k^T/8) v
         -> out @ W_out[1024, 1024].

Sharding (8 cores): batch (2-way) x head-group (4-way, 4 heads each).
Each core computes a partial output [2048, 1024] = attnout_heads @ W_out_rows;
host sums the 4 head-group partials per batch.

v2 design (vs baseline): single fused pipeline instead of serial phases.
 - x is DMA'd in 512-column chunks; k/v/q projections start on chunk 0
   while later chunks stream in.
 - Attention inner loop is ACT(exp)-bound (~1.15us per sk tile); leftover
   projection + output-projection matmuls are interleaved ("fill work")
   into the PE slack of the attention loop.
 - Scores use K=64 matmuls on packed q/k tiles (measured same throughput
   as K=128 on f32r/bf16; no zero-padded q needed).
 - All attention operands (kT, q, exp(attn), v, att_o, W_out) are bf16:
   same PE rate, half SBUF, 2x DVE. Projections stay f32r from f32 x/W.
 - Softmax denominator rides as a ones-column in the attnv lhsT (row 64
   of the psum accumulator); normalization = reciprocal of that row,
   K=1 ones-outer-product broadcast matmul, one DVE multiply.
 - PSUM: scores double-buffered (2x2 banks) + attnv accumulator (2 banks)
   + a shared 1-bank ring (proj/rope/broadcast/outproj) x2.
"""

import sys

if "/opt/trn_rl_repo" not in sys.path:
    sys.path.insert(0, "/opt/trn_rl_repo")

import numpy as np

B, S, E = 2, 2048, 1024
ATT = 1024
H = 16
D = 64
HG = 4            # head groups (cores per batch)
HPG = H // HG     # heads per core = 4
PAIRS = HPG // 2  # head pairs per core = 2
ROPE_THETA = 10000.0
N_CORES = 8

EK = E // 128     # 8 contraction tiles over embedding dim
NCH = S // 512    # 4 x/proj column chunks
N_SK = S // 128   # 16 sk tiles
SQ_CHUNK = 1024   # q chunk for attention blocks
N_CH = S // SQ_CHUNK  # 2 attention chunks

_BUILT = {}


def _build_program():
    import concourse.bacc as bacc
    import concourse.tile as tile
    import concourse.mybir as mybir

    f32 = mybir.dt.float32
    f32r = mybir.dt.float32r
    bf16 = mybir.dt.bfloat16
    AF = mybir.ActivationFunctionType

    nc = bacc.Bacc(
        "TRN2",
        target_bir_lowering=False,
        debug=False,
        enable_asserts=False,
        num_devices=N_CORES,
    )

    xT = nc.dram_tensor("xT", [E, S], f32r, kind="ExternalInput").ap()
    w_qk = nc.dram_tensor("w_qk", [E, 2 * HPG * D], f32r, kind="ExternalInput").ap()
    w_v = nc.dram_tensor("w_v", [E, HPG * D], f32r, kind="ExternalInput").ap()
    w_o = nc.dram_tensor("w_o", [HPG * D, E], bf16, kind="ExternalInput").ap()
    cos_t = nc.dram_tensor("cos_t", [128, S], f32, kind="ExternalInput").ap()
    sin_t = nc.dram_tensor("sin_t", [128, S], f32, kind="ExternalInput").ap()
    mswap = nc.dram_tensor("mswap", [128, 128], f32r, kind="ExternalInput").ap()
    out = nc.dram_tensor("out", [S, E], f32, kind="ExternalOutput").ap()

    with tile.TileContext(nc) as tc:
        with (
            tc.tile_pool(name="const", bufs=1) as constp,
            tc.tile_pool(name="persist", bufs=1) as pers,
            tc.tile_pool(name="xt", bufs=1) as xtp,
            tc.tile_pool(name="wqk", bufs=1) as wqkp,
            tc.tile_pool(name="wv", bufs=1) as wvp,
            tc.tile_pool(name="trig", bufs=1) as trigp,
            tc.tile_pool(name="raw", bufs=3) as rawp,
            tc.tile_pool(name="tt", bufs=4) as ttp,
            tc.tile_pool(name="exp", bufs=3) as expp,
            tc.tile_pool(name="oa", bufs=2) as oap,
            tc.tile_pool(name="ev", bufs=3) as evp,
            tc.tile_pool(name="sA", bufs=2, space="PSUM") as sAp,
            tc.tile_pool(name="oT", bufs=1, space="PSUM") as oTp,
            tc.tile_pool(name="ppp", bufs=1, space="PSUM") as ppp,
            tc.tile_pool(name="r1", bufs=1, space="PSUM") as r1p,
        ):
            msw = constp.tile([128, 128], f32r, tag="msw")
            # (f32r/bf16 memsets are invalid ISA; memset f32 and copy.)
            ones_f32 = constp.tile([128, 64], f32, tag="ones_f32")
            nc.gpsimd.memset(ones_f32[:], 1.0)

            # persistent attention tensors (all bf16)
            kT = [pers.tile([128, S], bf16, tag=f"kT{g}", name=f"kT{g}") for g in range(PAIRS)]
            qp = [pers.tile([128, S], bf16, tag=f"qp{g}", name=f"qp{g}") for g in range(PAIRS)]
            # v + ones-aug col per (pair g, head hh): cols 130g+65hh .. +65
            v_c = pers.tile([128, N_SK, 4 * 65], bf16, tag="vc")
            for hcol in range(4):
                nc.vector.tensor_copy(
                    v_c[:, :, 65 * hcol + 64], ones_f32[:, 0:N_SK]
                )
            att_o = [pers.tile([128, S], bf16, tag=f"ao{g}", name=f"ao{g}") for g in range(PAIRS)]
            wo_sb = [pers.tile([128, E], bf16, tag=f"wo{g}", name=f"wo{g}") for g in range(PAIRS)]

            cos_sb = trigp.tile([128, S], f32, tag="cos")
            sin_sb = trigp.tile([128, S], f32, tag="sin")

            # ---------------- DMA (ordered; x chunked so compute starts early)
            wqk_sb = []
            for e in range(EK):
                t = wqkp.tile([128, 2 * HPG * D], f32r, tag=f"wqk{e}", name=f"wqk{e}")
                nc.sync.dma_start(t[:], w_qk[128 * e : 128 * (e + 1), :])
                wqk_sb.append(t)
            xt_sb = [[None] * NCH for _ in range(EK)]
            for c in range(NCH):
                csl = slice(512 * c, 512 * (c + 1))
                for e in range(EK):
                    t = xtp.tile([128, 512], f32r, tag=f"xt{e}_{c}", name=f"xt{e}_{c}")
                    nc.sync.dma_start(t[:], xT[128 * e : 128 * (e + 1), csl])
                    xt_sb[e][c] = t
                nc.sync.dma_start(cos_sb[:, csl], cos_t[:, csl])
                nc.sync.dma_start(sin_sb[:, csl], sin_t[:, csl])
                if c == 0:
                    nc.sync.dma_start(msw[:], mswap[:])
                    wv_sb = []
                    for e in range(EK):
                        tv = wvp.tile([128, HPG * D], f32r, tag=f"wv{e}", name=f"wv{e}")
                        nc.sync.dma_start(tv[:], w_v[128 * e : 128 * (e + 1), :])
                        wv_sb.append(tv)
                if c == 1:
                    for g in range(PAIRS):
                        nc.sync.dma_start(wo_sb[g][:], w_o[128 * g : 128 * (g + 1), :])

            # ---------------- projection + rope helpers ----------------
            rope_pend = []

            def rope_tail():
                if not rope_pend:
                    return
                (g_, ti, sl, raw) = rope_pend.pop(0)
                rp = r1p.tile([128, 512], f32, tag="r1", name=f"rot{g_}_{ti}_{sl.start}")
                nc.tensor.matmul(rp[:], msw[:], raw[:], start=True, stop=True)
                t2 = ttp.tile([128, 512], f32, tag="tt")
                nc.vector.tensor_mul(t2[:], raw[:], cos_sb[:, sl])
                t1 = ttp.tile([128, 512], f32, tag="tt")
                nc.vector.tensor_mul(t1[:], rp[:], sin_sb[:, sl])
                if ti == 1:
                    nc.vector.tensor_add(kT[g_][:, sl], t1[:], t2[:])
                else:
                    nc.gpsimd.tensor_tensor(
                        qp[g_][:, sl], t1[:], t2[:], mybir.AluOpType.add
                    )

            def qk_items(g, ti, c):
                """Items projecting+roping one 512-col chunk of q (ti=0) or
                k (ti=1) of pair g: 8 single-matmul items + raw-copy + rope."""
                coff = ti * HPG * D + 128 * g
                sl = slice(512 * c, 512 * (c + 1))
                cell = {}

                def mk_mm(e):
                    def it():
                        if e == 0:
                            cell["pp"] = ppp.tile(
                                [128, 512], f32, tag="pp", name=f"pj{g}_{ti}_{c}"
                            )
                        nc.tensor.matmul(
                            cell["pp"][:],
                            wqk_sb[e][:, coff : coff + 128],
                            xt_sb[e][c][:],
                            start=(e == 0),
                            stop=(e == EK - 1),
                        )
                    return it

                def raw_it():
                    raw = rawp.tile([128, 512], f32r, tag="raw")
                    nc.vector.tensor_copy(raw[:], cell["pp"][:])
                    rope_pend.append((g, ti, sl, raw))

                return [mk_mm(e) for e in range(EK)] + [raw_it, rope_tail]

            def qk_chunk(g, ti, c):
                for it in qk_items(g, ti, c):
                    it()

            def v_items(st):
                c, sub = divmod(st, 4)
                cell = {}

                def mk_mm(e):
                    def it():
                        if e == 0:
                            cell["vp"] = r1p.tile(
                                [128, 256], f32, tag="r1", name=f"vps{st}"
                            )
                        nc.tensor.matmul(
                            cell["vp"][:],
                            xt_sb[e][c][:, 128 * sub : 128 * (sub + 1)],
                            wv_sb[e][:],
                            start=(e == 0),
                            stop=(e == EK - 1),
                        )
                    return it

                def cp_it():
                    for h in range(4):
                        nc.vector.tensor_copy(
                            v_c[:, st, 65 * h : 65 * h + 64],
                            cell["vp"][:, 64 * h : 64 * h + 64],
                        )

                return [mk_mm(e) for e in range(EK)] + [cp_it]

            def v_st(st):
                for it in v_items(st):
                    it()

            # ---------------- preamble (paced by x-chunk arrival) ----------
            # Minimum before attention can start: kT pair 0 (all chunks),
            # v (all, consumed 1 tile/iter by attnv), q pair 0 cols 0:1024.
            for c in range(NCH):
                qk_chunk(0, 1, c)            # k pair 0
                for st in range(4 * c, 4 * c + 4):
                    v_st(st)
                if c < 2:
                    qk_chunk(0, 0, c)        # q pair 0 chunks 0,1
            rope_tail()                      # flush pending rope
            rope_tail()

            # fill work queue: pumped a few items per attention iteration
            fill = []
            for (g_, ti_, c_) in [(0, 0, 2), (0, 0, 3),
                                  (1, 1, 0), (1, 1, 1), (1, 1, 2), (1, 1, 3),
                                  (1, 0, 0), (1, 0, 1), (1, 0, 2), (1, 0, 3)]:
                fill.extend(qk_items(g_, ti_, c_))
            fill.append(rope_tail)  # drain last pending rope
            fill.append(rope_tail)

            def out_st(st, n):
                ssl = slice(128 * st, 128 * (st + 1))
                nsl = slice(512 * n, 512 * (n + 1))
                op = r1p.tile([128, 512], f32, tag="r1", name=f"op{st}_{n}")
                for g in range(PAIRS):
                    nc.tensor.matmul(
                        op[:],
                        att_o[g][:, ssl],
                        wo_sb[g][:, nsl],
                        start=(g == 0),
                        stop=(g == PAIRS - 1),
                    )
                ev = evp.tile([128, 512], f32, tag="ev")
                nc.vector.tensor_copy(ev[:], op[:])
                nc.sync.dma_start(out[ssl, nsl], ev[:])

            # ---------------- attention (ACT-bound; PE slack runs fill) ----
            def attention_block(g, ch, hh):
                base = 130 * g + 65 * hh
                rows = slice(64 * hh, 64 * hh + 64)
                csl = slice(SQ_CHUNK * ch, SQ_CHUNK * (ch + 1))
                oT = oTp.tile([65, SQ_CHUNK], f32, tag="oT", name=f"oT{g}_{ch}_{hh}")
                exps = []

                def attnv(j):
                    e_t = exps[j]
                    for n in range(2):
                        nsl = slice(512 * n, 512 * (n + 1))
                        nc.tensor.matmul(
                            oT[:, nsl],
                            v_c[:, j, base : base + 65],
                            e_t[:, nsl],
                            start=(j == 0),
                            stop=(j == N_SK - 1),
                        )

                for sk in range(N_SK):
                    sksl = slice(128 * sk, 128 * (sk + 1))
                    sA = sAp.tile([128, SQ_CHUNK], f32, tag="sA",
                                  name=f"sA{g}_{ch}_{hh}_{sk}")
                    for n in range(2):
                        nsl = slice(512 * n, 512 * (n + 1))
                        gsl = slice(SQ_CHUNK * ch + 512 * n,
                                    SQ_CHUNK * ch + 512 * (n + 1))
                        nc.tensor.matmul(
                            sA[:, nsl],
                            kT[g][rows, sksl],
                            qp[g][rows, gsl],
                            start=True,
                            stop=True,
                        )
                    e_t = expp.tile([128, SQ_CHUNK], bf16, tag="eA")
                    nc.scalar.activation(e_t[:], sA[:], AF.Exp, scale=0.125)
                    exps.append(e_t)
                    if sk > 0:
                        attnv(sk - 1)
                    npump = 2 if nblk[0] < 2 else 1
                    for _ in range(npump):
                        if fill:
                            fill.pop(0)()
                attnv(N_SK - 1)
                nblk[0] += 1

                # normalize: denom in oT row 64. Copy to SBUF promptly
                # (frees oT for the next block); the broadcast/reciprocal/
                # multiply are queued as fill work so the block boundary
                # doesn't serialize the scores/exp pipeline.
                oA = oap.tile([64, SQ_CHUNK], f32, tag="oA")
                nc.vector.tensor_copy(oA[:], oT[0:64, :])
                dn = oap.tile([1, SQ_CHUNK], f32, tag="dn")
                nc.vector.tensor_copy(dn[:], oT[64:65, :])

                rbb = oap.tile([64, SQ_CHUNK], f32, tag="rbb")
                nc.gpsimd.partition_broadcast(rbb[:], dn[:])
                rbr = oap.tile([64, SQ_CHUNK], f32, tag="rbr")
                nc.vector.reciprocal_approx_fast(rbr[:], rbb[:])
                nc.vector.tensor_mul(
                    att_o[g][rows, csl], oA[:], rbr[:]
                )

            iters_left = [8 * N_SK]

            nblk = [0]
            for g in range(PAIRS):
                for ch in range(N_CH):
                    for hh in range(2):
                        attention_block(g, ch, hh)
                    if g == 1:
                        for st in range(8 * ch, 8 * ch + 8):
                            fill.append(lambda st=st: out_st(st, 0))
                            fill.append(lambda st=st: out_st(st, 1))

            while fill:
                fill.pop(0)()

    nc.compile()
    return nc


def _get_program():
    if "nc" not in _BUILT:
        _BUILT["nc"] = _build_program()
    return _BUILT["nc"]


def _host_inputs(x, W_qkv, W_out):
    """Build the 8 per-core input maps."""
    import ml_dtypes

    f = np.float32
    x = np.asarray(x, dtype=f)
    W_qkv = np.asarray(W_qkv, dtype=f)
    W_out = np.asarray(W_out, dtype=f)

    inv_freq = 1.0 / (ROPE_THETA ** (np.arange(0, D, 2, dtype=np.float64) / D))
    p = np.arange(128)
    freq_row = inv_freq[(p % D) // 2]  # [128]
    ang = freq_row[:, None] * np.arange(S, dtype=np.float64)[None, :]  # [128, S]
    cos_t = np.cos(ang).astype(f)
    sign = np.where(p % 2 == 0, -1.0, 1.0)[:, None]
    sin_t = (np.sin(ang) * sign).astype(f)

    msw = np.zeros((128, 128), dtype=f)
    msw[p, p ^ 1] = 1.0

    maps = []
    for core in range(N_CORES):
        b, hg = divmod(core, HG)
        hs = [HPG * hg + i for i in range(HPG)]
        w_qk = np.concatenate(
            [W_qkv[:, h * D : (h + 1) * D] for h in hs]
            + [W_qkv[:, ATT + h * D : ATT + (h + 1) * D] for h in hs],
            axis=1,
        )
        w_v = np.concatenate(
            [W_qkv[:, 2 * ATT + h * D : 2 * ATT + (h + 1) * D] for h in hs], axis=1
        )
        w_o = np.concatenate([W_out[h * D : (h + 1) * D, :] for h in hs], axis=0)
        maps.append(
            {
                "xT": np.ascontiguousarray(x[b].T),
                "w_qk": np.ascontiguousarray(w_qk),
                "w_v": np.ascontiguousarray(w_v),
                "w_o": np.ascontiguousarray(w_o).astype(ml_dtypes.bfloat16),
                "cos_t": cos_t,
                "sin_t": sin_t,
                "mswap": msw,
            }
        )
    return maps


def kernel(x, W_qkv, W_out):
    from concourse.bass_utils import run_bass_kernel_spmd

    nc = _get_program()
    maps = _host_inputs(x, W_qkv, W_out)
    res = run_bass_kernel_spmd(nc, maps, core_ids=list(range(N_CORES)))
    out = np.zeros((B, S, E), dtype=np.float32)
    for core in range(N_CORES):
        b = core // HG
        out[b] += res.results[core]["out"]
    return out
